# revision 17
# baseline (speedup 1.0000x reference)
"""BayesianDense (training path) Trainium2 kernel.

Computes, for B=512, D=512, O=256:
    sigma  = exp(W_log_sigma / 2)                     (D, O)
    out[b] = x[b] @ W_mu
           + sum_d x[b,d] * sigma[d,:] * e[b,d,:]     (noise matvec)
           + b_mu + eb[b] * exp(b_log_sigma / 2)

Data-parallel over batch across 8 NeuronCores (64 examples/core).
The dominant cost is streaming e (256 MB total, 32 MB/core) from HBM,
so the kernel is built to run at the HBM roofline:
  - e is DMAd in 4 MB chunks (8 examples) as [128, (b n o)] tiles
    (partition = d%128 within each 128-row d-block n).
  - DVE does one full-width tensor_mul per example: t = e_blk * sigma.
  - PE reduces over d: per (example, d-block) matvec with lhsT =
    x-column (128,1), rhs = t block (128,256), accumulated in PSUM.
    x@W_mu (M=8 per chunk) and the bias row (identity matmul) are
    folded into the same PSUM accumulation group.
  - ACT evacuates each chunk's (8,256) PSUM rows to SBUF.
"""

import numpy as np

B, D, O = 512, 512, 256
NCORES = 8
BL = B // NCORES          # 64 examples per core
P = 128                   # SBUF partitions
ND = D // P               # 4 d-blocks of 128
CHUNK = 8                 # examples per e-DMA chunk
NCHUNK = BL // CHUNK      # 8 chunks per core

# dtype for the per-example noise matvec matmuls:
#   "fp32r" -> single-pass fp32 (1 cyc/row at N>=256), slightly reduced precision
#   "fp32"  -> exact fp32 (4 cyc/row)
MATMUL_MODE = "fp32r"

_cache = {}


def _build(reps=1):
    import concourse.mybir as mybir
    import concourse.tile as tile
    from concourse import bacc

    f32 = mybir.dt.float32
    f32r = mybir.dt.float32r
    Exp = mybir.ActivationFunctionType.Exp

    nc = bacc.Bacc("TRN2", target_bir_lowering=False, debug=False,
                   num_devices=NCORES)

    e_d = nc.dram_tensor("e", [BL, D, O], f32, kind="ExternalInput").ap()
    xT_d = nc.dram_tensor("xT", [D, BL], f32, kind="ExternalInput").ap()
    wmu_d = nc.dram_tensor("W_mu", [D, O], f32, kind="ExternalInput").ap()
    wls_d = nc.dram_tensor("W_ls", [D, O], f32, kind="ExternalInput").ap()
    eb_d = nc.dram_tensor("eb", [BL, O], f32, kind="ExternalInput").ap()
    bmu_d = nc.dram_tensor("bmu64", [BL, O], f32, kind="ExternalInput").ap()
    bls_d = nc.dram_tensor("bls64", [BL, O], f32, kind="ExternalInput").ap()
    out_d = nc.dram_tensor("out", [BL, O], f32, kind="ExternalOutput").ap()

    with tile.TileContext(nc) as tc:
        with tc.tile_pool(name="const", bufs=1) as cpool, \
             tc.tile_pool(name="chunks", bufs=3) as chpool, \
             tc.tile_pool(name="prod", bufs=6) as tpool, \
             tc.tile_pool(name="psum", bufs=6, space="PSUM") as pspool, \
             tc.tile_pool(name="psum_wmu", bufs=1, space="PSUM") as pwpool:
          for _rep in range(reps):
            # ---- constants / params -------------------------------------
            # sigma, W_mu, xT live in [p, (n, ...)] layout: row-block n of
            # the D axis maps to free offset n*O (resp. n*BL).
            sigma = cpool.tile([P, ND * O], f32)
            nc.sync.dma_start(sigma[:].rearrange("p (n o) -> p n o", n=ND),
                              wls_d.rearrange("(n p) o -> p n o", p=P))
            nc.scalar.activation(sigma[:], sigma[:], Exp, scale=0.5)

            wmu = cpool.tile([P, ND * O], f32)
            nc.sync.dma_start(wmu[:].rearrange("p (n o) -> p n o", n=ND),
                              wmu_d.rearrange("(n p) o -> p n o", p=P))

            xT = cpool.tile([P, ND * BL], f32)
            nc.sync.dma_start(xT[:].rearrange("p (n b) -> p n b", n=ND),
                              xT_d.rearrange("(n p) b -> p n b", p=P))
            if MATMUL_MODE == "fp32r":
                # fp32r matmul operands must be produced rounded-to-fp32r
                xTr = cpool.tile([P, ND * BL], f32r)
                nc.vector.tensor_copy(xTr[:], xT[:])
            else:
                xTr = xT

            # bias[b, o] = b_mu[o] + eb[b, o] * exp(b_log_sigma[o] / 2)
            sigb = cpool.tile([BL, O], f32)
            nc.sync.dma_start(sigb[:], bls_d[:, :])
            nc.scalar.activation(sigb[:], sigb[:], Exp, scale=0.5)
            ebt = cpool.tile([BL, O], f32)
            nc.sync.dma_start(ebt[:], eb_d[:, :])
            bmu = cpool.tile([BL, O], f32)
            nc.sync.dma_start(bmu[:], bmu_d[:, :])
            bias = cpool.tile([BL, O], f32)
            nc.vector.tensor_mul(bias[:], ebt[:], sigb[:])
            nc.vector.tensor_add(bias[:], bias[:], bmu[:])

            # x @ W_mu for all 64 rows (exact fp32), evacuated to SBUF
            ps_wmu = pwpool.tile([BL, O], f32)
            for n in range(ND):
                nc.tensor.matmul(
                    ps_wmu[:, :],
                    lhsT=xT[:, n * BL:(n + 1) * BL],
                    rhs=wmu[:, n * O:(n + 1) * O],
                    start=(n == 0), stop=(n == ND - 1),
                )
            wmu_sb = cpool.tile([BL, O], f32)
            nc.scalar.copy(wmu_sb[:], ps_wmu[:, :])

            # per-example noise results accumulate into a partition-0 strip
            stage = cpool.tile([1, BL * O], f32)
            out_sb = cpool.tile([BL, O], f32)

            e_r = e_d.rearrange("(c b) (n p) o -> c p b n o", b=CHUNK, p=P)

            # ---- main loop ----------------------------------------------
            for c in range(NCHUNK):
                ch = chpool.tile([P, CHUNK * ND * O], f32)
                nc.sync.dma_start(
                    ch[:].rearrange("p (b n o) -> p b n o", b=CHUNK, n=ND),
                    e_r[c],
                )
                for b in range(CHUNK):
                    t = tpool.tile([P, ND * O],
                                   f32r if MATMUL_MODE == "fp32r" else f32)
                    nc.vector.tensor_mul(
                        t[:], ch[:, b * ND * O:(b + 1) * ND * O], sigma[:])
                    bg = c * CHUNK + b
                    ps = pspool.tile([1, O], f32)
                    for n in range(ND):
                        nc.tensor.matmul(
                            ps[:, :],
                            lhsT=xTr[:, n * BL + bg: n * BL + bg + 1],
                            rhs=t[:, n * O:(n + 1) * O],
                            start=(n == 0), stop=(n == ND - 1),
                        )
                    nc.scalar.copy(stage[:, bg * O:(bg + 1) * O], ps[:, :])

            # scatter the strip across partitions (DMA moves across lanes)
            noise_sb = cpool.tile([BL, O], f32)
            nc.sync.dma_start(
                noise_sb[:],
                stage[:].rearrange("one (b o) -> one b o", b=BL),
            )
            nc.vector.tensor_add(out_sb[:], noise_sb[:], wmu_sb[:])
            nc.vector.tensor_add(out_sb[:], out_sb[:], bias[:])

            nc.sync.dma_start(out_d[:, :], out_sb[:])

    nc.compile()
    return nc


def _get_nc(reps=1):
    key = ("nc", reps)
    if key not in _cache:
        _cache[key] = _build(reps)
    return _cache[key]


def _in_maps(x, W_mu, W_log_sigma, b_mu, b_log_sigma, e, eb):
    x = np.asarray(x, dtype=np.float32)
    W_mu = np.ascontiguousarray(W_mu, dtype=np.float32)
    W_ls = np.ascontiguousarray(W_log_sigma, dtype=np.float32)
    e = np.asarray(e, dtype=np.float32)
    eb = np.asarray(eb, dtype=np.float32)
    bmu64 = np.ascontiguousarray(np.broadcast_to(b_mu, (BL, O)), dtype=np.float32)
    bls64 = np.ascontiguousarray(np.broadcast_to(b_log_sigma, (BL, O)), dtype=np.float32)
    maps = []
    for c in range(NCORES):
        sl = slice(c * BL, (c + 1) * BL)
        maps.append({
            "e": np.ascontiguousarray(e[sl]),
            "xT": np.ascontiguousarray(x[sl].T),
            "W_mu": W_mu,
            "W_ls": W_ls,
            "eb": np.ascontiguousarray(eb[sl]),
            "bmu64": bmu64,
            "bls64": bls64,
        })
    return maps


def run(trace=False, **inputs):
    """Run on the 8 NeuronCores; returns (full_output, BassKernelResults)."""
    from concourse.bass_utils import run_bass_kernel_spmd

    nc = _get_nc()
    maps = _in_maps(**inputs)
    res = run_bass_kernel_spmd(nc, maps, list(range(NCORES)), trace=trace)
    out = np.concatenate([r["out"] for r in res.results], axis=0)
    return out, res


def kernel(**inputs) -> np.ndarray:
    out, _ = run(trace=False, **inputs)
    return out


# revision 20
# speedup vs baseline: 1.5540x; 1.5540x over previous
"""BayesianDense (training path) Trainium2 kernel.

Computes, for B=512, D=512, O=256:
    sigma  = exp(W_log_sigma / 2)                     (D, O)
    out[b] = x[b] @ W_mu
           + sum_d x[b,d] * sigma[d,:] * e[b,d,:]     (noise matvec)
           + b_mu + eb[b] * exp(b_log_sigma / 2)

Data-parallel over batch across 8 NeuronCores (64 examples/core).
The dominant cost is streaming e (256 MB total, 32 MB/core) from HBM,
so the kernel is built to run at the HBM roofline:
  - e is DMAd in 4 MB chunks (8 examples) as [128, (b n o)] tiles
    (partition = d%128 within each 128-row d-block n).
  - DVE does one full-width tensor_mul per example: t = e_blk * sigma.
  - PE reduces over d: per (example, d-block) matvec with lhsT =
    x-column (128,1), rhs = t block (128,256), accumulated in PSUM.
    x@W_mu (M=8 per chunk) and the bias row (identity matmul) are
    folded into the same PSUM accumulation group.
  - ACT evacuates each chunk's (8,256) PSUM rows to SBUF.
"""

import numpy as np

B, D, O = 512, 512, 256
NCORES = 8
BL = B // NCORES          # 64 examples per core
P = 128                   # SBUF partitions
ND = D // P               # 4 d-blocks of 128
CHUNK = 8                 # examples per e-DMA chunk
NCHUNK = BL // CHUNK      # 8 chunks per core

# dtype for the per-example noise matvec matmuls:
#   "fp32r" -> single-pass fp32 (1 cyc/row at N>=256), slightly reduced precision
#   "fp32"  -> exact fp32 (4 cyc/row)
MATMUL_MODE = "fp32r"

_cache = {}


def _build(reps=1):
    import concourse.mybir as mybir
    import concourse.tile as tile
    from concourse import bacc

    f32 = mybir.dt.float32
    f32r = mybir.dt.float32r
    Exp = mybir.ActivationFunctionType.Exp

    nc = bacc.Bacc("TRN2", target_bir_lowering=False, debug=False,
                   num_devices=NCORES)

    e_d = nc.dram_tensor("e", [BL, D, O], f32, kind="ExternalInput").ap()
    xT_d = nc.dram_tensor("xT", [D, BL], f32, kind="ExternalInput").ap()
    wmu_d = nc.dram_tensor("W_mu", [D, O], f32, kind="ExternalInput").ap()
    wls_d = nc.dram_tensor("W_ls", [D, O], f32, kind="ExternalInput").ap()
    eb_d = nc.dram_tensor("eb", [BL, O], f32, kind="ExternalInput").ap()
    bmu_d = nc.dram_tensor("bmu64", [BL, O], f32, kind="ExternalInput").ap()
    bls_d = nc.dram_tensor("bls64", [BL, O], f32, kind="ExternalInput").ap()
    out_d = nc.dram_tensor("out", [BL, O], f32, kind="ExternalOutput").ap()

    with tile.TileContext(nc) as tc:
        with tc.tile_pool(name="const", bufs=1) as cpool, \
             tc.tile_pool(name="chunks", bufs=3) as chpool, \
             tc.tile_pool(name="prod", bufs=6) as tpool, \
             tc.tile_pool(name="psum", bufs=6, space="PSUM") as pspool, \
             tc.tile_pool(name="psum_wmu", bufs=1, space="PSUM") as pwpool:
          for _rep in range(reps):
            # ---- constants / params -------------------------------------
            # Flat layout: the D axis splits as d = 4*a + j with a the SBUF
            # partition and (j, o) the free dims — every DMA then moves 4 KB
            # contiguous runs per partition. Const loads ride the SWDGE
            # (gpsimd) queue to keep both HWDGE rings free for the e-stream.
            sigma = cpool.tile([P, ND * O], f32)
            nc.gpsimd.dma_start(sigma[:].rearrange("a (j o) -> a j o", j=ND),
                                wls_d.rearrange("(a j) o -> a j o", a=P))
            nc.scalar.activation(sigma[:], sigma[:], Exp, scale=0.5)

            wmu = cpool.tile([P, ND * O], f32)
            nc.gpsimd.dma_start(wmu[:].rearrange("a (j o) -> a j o", j=ND),
                                wmu_d.rearrange("(a j) o -> a j o", a=P))

            xT = cpool.tile([P, ND * BL], f32)
            nc.gpsimd.dma_start(xT[:].rearrange("a (j b) -> a j b", j=ND),
                                xT_d.rearrange("(a j) b -> a j b", a=P))
            if MATMUL_MODE == "fp32r":
                # fp32r matmul operands must be produced rounded-to-fp32r
                xTr = cpool.tile([P, ND * BL], f32r)
                nc.vector.tensor_copy(xTr[:], xT[:])
            else:
                xTr = xT

            # bias[b, o] = b_mu[o] + eb[b, o] * exp(b_log_sigma[o] / 2)
            sigb = cpool.tile([BL, O], f32)
            nc.gpsimd.dma_start(sigb[:], bls_d[:, :])
            nc.scalar.activation(sigb[:], sigb[:], Exp, scale=0.5)
            ebt = cpool.tile([BL, O], f32)
            nc.gpsimd.dma_start(ebt[:], eb_d[:, :])
            bmu = cpool.tile([BL, O], f32)
            nc.gpsimd.dma_start(bmu[:], bmu_d[:, :])
            bias = cpool.tile([BL, O], f32)
            nc.vector.tensor_mul(bias[:], ebt[:], sigb[:])
            nc.vector.tensor_add(bias[:], bias[:], bmu[:])

            # x @ W_mu for all 64 rows (exact fp32), evacuated to SBUF
            ps_wmu = pwpool.tile([BL, O], f32)
            for n in range(ND):
                nc.tensor.matmul(
                    ps_wmu[:, :],
                    lhsT=xT[:, n * BL:(n + 1) * BL],
                    rhs=wmu[:, n * O:(n + 1) * O],
                    start=(n == 0), stop=(n == ND - 1),
                )
            wmu_sb = cpool.tile([BL, O], f32)
            nc.scalar.copy(wmu_sb[:], ps_wmu[:, :])

            # per-example noise results accumulate into a partition-0 strip
            stage = cpool.tile([1, BL * O], f32)
            out_sb = cpool.tile([BL, O], f32)

            e_r = e_d.rearrange("(c b) (a j) o -> c a b j o", b=CHUNK, a=P)

            # ---- main loop ----------------------------------------------
            for c in range(NCHUNK):
                ch = chpool.tile([P, CHUNK * ND * O], f32)
                chv = ch[:].rearrange("a (b j o) -> a b j o", b=CHUNK, j=ND)
                half = CHUNK // 2
                # split each chunk across the two HWDGE rings (SP + ACT)
                nc.sync.dma_start(chv[:, :half], e_r[c][:, :half])
                nc.scalar.dma_start(chv[:, half:], e_r[c][:, half:])
                for b in range(CHUNK):
                    t = tpool.tile([P, ND * O],
                                   f32r if MATMUL_MODE == "fp32r" else f32)
                    nc.vector.tensor_mul(
                        t[:], ch[:, b * ND * O:(b + 1) * ND * O], sigma[:])
                    bg = c * CHUNK + b
                    ps = pspool.tile([1, O], f32)
                    for n in range(ND):
                        nc.tensor.matmul(
                            ps[:, :],
                            lhsT=xTr[:, n * BL + bg: n * BL + bg + 1],
                            rhs=t[:, n * O:(n + 1) * O],
                            start=(n == 0), stop=(n == ND - 1),
                        )
                    nc.scalar.copy(stage[:, bg * O:(bg + 1) * O], ps[:, :])

            # scatter the strip across partitions (DMA moves across lanes)
            noise_sb = cpool.tile([BL, O], f32)
            nc.sync.dma_start(
                noise_sb[:],
                stage[:].rearrange("one (b o) -> one b o", b=BL),
            )
            nc.vector.tensor_add(out_sb[:], noise_sb[:], wmu_sb[:])
            nc.vector.tensor_add(out_sb[:], out_sb[:], bias[:])

            nc.sync.dma_start(out_d[:, :], out_sb[:])

    nc.compile()
    return nc


def _get_nc(reps=1):
    key = ("nc", reps)
    if key not in _cache:
        _cache[key] = _build(reps)
    return _cache[key]


def _in_maps(x, W_mu, W_log_sigma, b_mu, b_log_sigma, e, eb):
    x = np.asarray(x, dtype=np.float32)
    W_mu = np.ascontiguousarray(W_mu, dtype=np.float32)
    W_ls = np.ascontiguousarray(W_log_sigma, dtype=np.float32)
    e = np.asarray(e, dtype=np.float32)
    eb = np.asarray(eb, dtype=np.float32)
    bmu64 = np.ascontiguousarray(np.broadcast_to(b_mu, (BL, O)), dtype=np.float32)
    bls64 = np.ascontiguousarray(np.broadcast_to(b_log_sigma, (BL, O)), dtype=np.float32)
    maps = []
    for c in range(NCORES):
        sl = slice(c * BL, (c + 1) * BL)
        maps.append({
            "e": np.ascontiguousarray(e[sl]),
            "xT": np.ascontiguousarray(x[sl].T),
            "W_mu": W_mu,
            "W_ls": W_ls,
            "eb": np.ascontiguousarray(eb[sl]),
            "bmu64": bmu64,
            "bls64": bls64,
        })
    return maps


def run(trace=False, **inputs):
    """Run on the 8 NeuronCores; returns (full_output, BassKernelResults)."""
    from concourse.bass_utils import run_bass_kernel_spmd

    nc = _get_nc()
    maps = _in_maps(**inputs)
    res = run_bass_kernel_spmd(nc, maps, list(range(NCORES)), trace=trace)
    out = np.concatenate([r["out"] for r in res.results], axis=0)
    return out, res


def kernel(**inputs) -> np.ndarray:
    out, _ = run(trace=False, **inputs)
    return out
